# revision 1
# baseline (speedup 1.0000x reference)
"""MMD loss kernel for Trainium2, SPMD across 8 NeuronCores.

Math: loss = (1/B^2) * sum_{ij} s_i s_j K_ij over the [2B, 2B] Gaussian
kernel-sum matrix, s = [+1]*B ++ [-1]*B.  K_ij = sum_{k=0..4} exp(-l2_ij / (bw*2^k))
with bw = mean off-diagonal l2 / 4 (computed on host via the algebraic identity
sum(l2) = 2N*sum(sq) - 2*||sum x||^2).

Device strategy per core (SPMD, identical program; per-core data sliced on host):
  - 16x16 grid of 512-wide blocks over the symmetric 8192x8192 matrix.
    Core c owns block-rows {2c, 2c+1}; each row computes wrapped diagonals
    d=0..8 (cols (row+d) mod 16). Weights (host-side): d=0 -> 1, d=1..7 -> 2,
    d=8 -> 1 (each d=8 pair computed from both sides).
  - Per block (512x512): one 4-bank PSUM group [128, 4, 512] accumulates -l2
    directly via an augmented GEMM: 8 bf16 k-tiles of (2*x_i)·(x_j) plus one
    K=2 fp32r matmul adding (-sq_i - sq_j) exactly.
  - ACT (wide passes from PSUM with fused row-sums): t4 = exp(4*c4*psum)
    then t1 = exp(c4*psum).  DVE (wide scalar_tensor_tensor squarings with
    fused row-sums): t8 = t4*t4, t16 = t8*t8, and sum(t^2) from t1*t1.
    The final block computes t2 on ACT instead so the drain tail is short.
  - Host reduces the [128, 90] per-core level-sums with block weights/signs.
"""

import sys

sys.path.insert(0, "/opt/trn_rl_repo")

import numpy as np
import ml_dtypes

import concourse.mybir as mybir
import concourse.tile as tile
from concourse import bacc
from concourse.bass_utils import run_bass_kernel_spmd

B = 4096
D = 1024
N = 2 * B
NB = 16          # block grid (512-wide)
BS = 512
KD = 8           # feature k-tiles of 128
NDIAG = 9        # wrapped diagonals per block-row
NCORES = 8
NBLK = 2 * NDIAG       # 18 blocks per core
NSLOT = NBLK * 5 + 2   # 5 level-sums per block (+2 extra for the split last block)

BF16 = mybir.dt.bfloat16
F32 = mybir.dt.float32
F32R = mybir.dt.float32r

_prog_cache = {}


def build_program():
    if "nc" in _prog_cache:
        return _prog_cache["nc"]
    nc = bacc.Bacc("TRN2", target_bir_lowering=False, debug=False, num_devices=NCORES)
    u_d = nc.dram_tensor("u", [2, 128, KD, BS], BF16, kind="ExternalInput").ap()
    v_d = nc.dram_tensor("v", [10, 128, KD, BS], BF16, kind="ExternalInput").ap()
    ua_d = nc.dram_tensor("ua", [128, 2, BS], F32R, kind="ExternalInput").ap()
    va_d = nc.dram_tensor("va", [128, 10, BS], F32R, kind="ExternalInput").ap()
    sc_d = nc.dram_tensor("sc", [128, 3], F32, kind="ExternalInput").ap()
    out_d = nc.dram_tensor("out", [128, NSLOT], F32, kind="ExternalOutput").ap()

    MULT = mybir.AluOpType.mult
    EXP = mybir.ActivationFunctionType.Exp

    with tile.TileContext(nc) as tc:
        with (
            tc.tile_pool(name="ustat", bufs=1) as upool,
            tc.tile_pool(name="vstat", bufs=1) as vpool,
            tc.tile_pool(name="aug", bufs=1) as augpool,
            tc.tile_pool(name="cst", bufs=1) as cstpool,
            tc.tile_pool(name="ot", bufs=1) as opool,
            tc.tile_pool(name="texp", bufs=2) as tpool,
            tc.tile_pool(name="wsq", bufs=2) as wpool,
            tc.tile_pool(name="ps", bufs=2, space="PSUM") as pspool,
        ):
            u_sb = upool.tile([128, 2, KD, BS], BF16)
            v_sb = vpool.tile([128, 10, KD, BS], BF16)
            ua_sb = augpool.tile([128, 2, BS], F32R)
            va_sb = augpool.tile([128, 10, BS], F32R)
            sc_sb = cstpool.tile([128, 3], F32)
            out_sb = opool.tile([128, NSLOT], F32)

            # Fine-grained interleave in consumption order: the first block
            # (r=0, d=0) gets per-kd chunks so the PE unblocks as early as the
            # sync queue's ~7us framework preamble allows (scalar/gpsimd DMA
            # queues were tried and are slower - software DGE path).
            for kd in range(KD):
                nc.sync.dma_start(out=u_sb[:, 0, kd], in_=u_d[0, :, kd])
                nc.sync.dma_start(out=v_sb[:, 0, kd], in_=v_d[0, :, kd])
                if kd == 2:
                    nc.sync.dma_start(out=sc_sb[:], in_=sc_d[:])
                    nc.sync.dma_start(out=ua_sb[:], in_=ua_d[:])
                    nc.sync.dma_start(out=va_sb[:], in_=va_d[:])
            for kd in range(KD):
                nc.sync.dma_start(out=v_sb[:, 1, kd], in_=v_d[1, :, kd])
            for s in range(2, 6):
                nc.sync.dma_start(out=v_sb[:, s], in_=v_d[s])
            nc.sync.dma_start(out=u_sb[:, 1], in_=u_d[1])
            for s in range(6, 10):
                nc.sync.dma_start(out=v_sb[:, s], in_=v_d[s])

            for r in range(2):
                for d in range(NDIAG):
                    blk = r * NDIAG + d
                    sbase = blk * 5
                    ps = pspool.tile([128, 4, BS], F32, name=f"ps_{r}_{d}", tag="ps")
                    for kd in range(KD):
                        for it in range(4):
                            nc.tensor.matmul(
                                ps[:, it, :],
                                lhsT=u_sb[:, r, kd, it * 128:(it + 1) * 128],
                                rhs=v_sb[:, r + d, kd, :],
                                start=(kd == 0),
                                stop=False,
                            )
                    # K=2 aug matmuls row-packed at 32-row strips: all four
                    # run concurrently on the PE (different row groups).
                    for it in range(4):
                        nc.tensor.matmul(
                            ps[:, it, :],
                            lhsT=ua_sb[32 * it:32 * it + 2, r, it * 128:(it + 1) * 128],
                            rhs=va_sb[32 * it:32 * it + 2, r + d, :],
                            start=False,
                            stop=True,
                            tile_position=(32 * it, 0),
                        )
                    last = blk == 2 * NDIAG - 1
                    if not last:
                        t4 = tpool.tile([128, 4, BS], BF16, name=f"t4_{blk}", tag="t4", bufs=3)
                        t1 = tpool.tile([128, 4, BS], F32, name=f"t1_{blk}", tag="t1")
                        # t4 first: it feeds the DVE chain, so DVE starts early.
                        nc.scalar.activation(
                            t4[:], ps[:, :, :], EXP,
                            scale=sc_sb[:, 2:3],
                            accum_out=out_sb[:, sbase + 2:sbase + 3],
                        )
                        nc.scalar.activation(
                            t1[:], ps[:, :, :], EXP,
                            scale=sc_sb[:, 0:1],
                            accum_out=out_sb[:, sbase:sbase + 1],
                        )
                        t8 = wpool.tile([128, 4, BS], BF16, name=f"t8_{blk}", tag="t8", bufs=3)
                        t16 = wpool.tile([128, 4, BS], BF16, name=f"t16_{blk}", tag="t16")
                        nc.vector.scalar_tensor_tensor(
                            out=t8[:], in0=t4[:], scalar=1.0, in1=t4[:],
                            op0=MULT, op1=MULT,
                            accum_out=out_sb[:, sbase + 3:sbase + 4],
                        )
                        nc.vector.scalar_tensor_tensor(
                            out=t16[:], in0=t8[:], scalar=1.0, in1=t8[:],
                            op0=MULT, op1=MULT,
                            accum_out=out_sb[:, sbase + 4:sbase + 5],
                        )
                        sq2 = wpool.tile([128, 4, BS], BF16, name=f"sq2_{blk}", tag="sq2")
                        nc.vector.scalar_tensor_tensor(
                            out=sq2[:], in0=t1[:], scalar=1.0, in1=t1[:],
                            op0=MULT, op1=MULT,
                            accum_out=out_sb[:, sbase + 1:sbase + 2],
                        )
                    else:
                        # Last block: split elementwise work in halves and put
                        # t2 on ACT so the post-matmul drain tail is short.
                        t4 = tpool.tile([128, 4, BS], BF16, name=f"t4_{blk}", tag="t4", bufs=3)
                        t1 = tpool.tile([128, 4, BS], F32, name=f"t1_{blk}", tag="t1")
                        t2 = tpool.tile([128, 4, BS], F32, name=f"t2_{blk}", tag="t2", bufs=1)
                        t8 = wpool.tile([128, 4, BS], BF16, name=f"t8_{blk}", tag="t8", bufs=3)
                        t16 = wpool.tile([128, 4, BS], BF16, name=f"t16_{blk}", tag="t16")
                        outcol = [out_sb[:, sbase + k:sbase + k + 1] for k in range(5)]
                        for h in range(2):
                            hs = slice(2 * h, 2 * h + 2)
                            nc.scalar.activation(
                                t4[:, hs, :], ps[:, hs, :], EXP,
                                scale=sc_sb[:, 2:3],
                                accum_out=outcol[2] if h == 0 else outcol[3],
                            )
                            nc.vector.scalar_tensor_tensor(
                                out=t8[:, hs, :], in0=t4[:, hs, :], scalar=1.0,
                                in1=t4[:, hs, :], op0=MULT, op1=MULT,
                                accum_out=outcol[4] if h == 0 else outcol[0],
                            )
                        # half sums are combined host-side; slots just need to
                        # hold them all (order within the block is irrelevant)
                        nc.scalar.activation(
                            t2[:], ps[:, :, :], EXP,
                            scale=sc_sb[:, 1:2],
                            accum_out=outcol[1],
                        )
                        nc.vector.scalar_tensor_tensor(
                            out=t16[:], in0=t8[:], scalar=1.0, in1=t8[:],
                            op0=MULT, op1=MULT,
                            accum_out=out_sb[:, sbase + 5:sbase + 6],
                        )
                        nc.scalar.activation(
                            t1[:], ps[:, :, :], EXP,
                            scale=sc_sb[:, 0:1],
                            accum_out=out_sb[:, sbase + 6:sbase + 7],
                        )
            nc.sync.dma_start(out=out_d[:], in_=out_sb[:])
    nc.compile()
    _prog_cache["nc"] = nc
    return nc


def prepare_inputs(source: np.ndarray, target: np.ndarray):
    """Host-side shard prep. Returns (in_maps, c4) for the 8 cores."""
    total = np.concatenate([source, target], axis=0).astype(np.float32)  # [N, D]
    t64 = total.astype(np.float64)
    sq64 = np.einsum("nd,nd->n", t64, t64)
    S1 = sq64.sum()
    vsum = t64.sum(axis=0)
    sum_l2 = 2.0 * N * S1 - 2.0 * (vsum @ vsum)
    bandwidth = sum_l2 / (N * N - N)
    bandwidth = bandwidth / (2.0 ** (5 // 2))  # KERNEL_MUL ** (KERNEL_NUM // 2)
    c4 = np.float64(1.0) / (16.0 * bandwidth)

    sq32 = sq64.astype(np.float32)
    Tt = np.ascontiguousarray(total.T)  # [D, N] f32
    u_all = (2.0 * Tt).astype(ml_dtypes.bfloat16).reshape(KD, 128, N)
    v_all = Tt.astype(ml_dtypes.bfloat16).reshape(KD, 128, N)

    sc_np = np.empty((128, 3), dtype=np.float32)
    sc_np[:, 0] = np.float32(c4)
    sc_np[:, 1] = np.float32(2.0 * c4)
    sc_np[:, 2] = np.float32(4.0 * c4)

    in_maps = []
    for c in range(NCORES):
        a0 = 2 * c
        u_np = np.empty((2, 128, KD, BS), dtype=ml_dtypes.bfloat16)
        ua_np = np.zeros((128, 2, BS), dtype=np.float32)
        for r in range(2):
            a = a0 + r
            cols = slice(a * BS, (a + 1) * BS)
            u_np[r] = u_all[:, :, cols].transpose(1, 0, 2)
            for g in range(4):
                ua_np[32 * g + 0, r] = -sq32[cols]
                ua_np[32 * g + 1, r] = -1.0
        v_np = np.empty((10, 128, KD, BS), dtype=ml_dtypes.bfloat16)
        va_np = np.zeros((128, 10, BS), dtype=np.float32)
        for s in range(10):
            g = (a0 + s) % NB
            cols = slice(g * BS, (g + 1) * BS)
            v_np[s] = v_all[:, :, cols].transpose(1, 0, 2)
            for gg in range(4):
                va_np[32 * gg + 0, s] = 1.0
                va_np[32 * gg + 1, s] = sq32[cols]
        in_maps.append(
            {"u": u_np, "v": v_np, "ua": ua_np, "va": va_np, "sc": sc_np}
        )
    return in_maps, c4


DIAG_W = np.array([1.0, 2.0, 2.0, 2.0, 2.0, 2.0, 2.0, 2.0, 1.0])


def reduce_outputs(outs):
    """outs: list of [128, NSLOT] f32 per core -> loss (np.float32 scalar)."""
    S = 0.0
    for c in range(NCORES):
        o = outs[c].astype(np.float64)  # [128, NSLOT]
        cols = o.sum(axis=0)  # [NSLOT]
        per_blk = cols[:NBLK * 5].reshape(2, NDIAG, 5).sum(axis=2)  # [2, 9]
        per_blk[1, NDIAG - 1] += cols[NBLK * 5:].sum()  # split last block extras
        for r in range(2):
            a = 2 * c + r
            sa = 1.0 if a < NB // 2 else -1.0
            for d in range(NDIAG):
                g = (a + d) % NB
                sg = 1.0 if g < NB // 2 else -1.0
                S += DIAG_W[d] * sa * sg * per_blk[r, d]
    return np.float32(S / (float(B) * float(B)))


def kernel(source: np.ndarray, target: np.ndarray) -> np.ndarray:
    nc = build_program()
    in_maps, _ = prepare_inputs(source, target)
    res = run_bass_kernel_spmd(nc, in_maps, list(range(NCORES)))
    outs = [res.results[c]["out"] for c in range(NCORES)]
    return np.asarray(reduce_outputs(outs), dtype=np.float32)

